# revision 15
# baseline (speedup 1.0000x reference)
"""Conformer (YMT3) relative-position self-attention on 8 Trainium2 cores.

Full-input contract: kernel(**inputs) takes the unsharded tensors and returns
(out, probs) exactly like the reference.

Sharding: core c handles batch (c % 4) and heads [8*(c//4), 8*(c//4)+8).
 - Wq/Wk/Wv/Wpos are sharded on their output dim (rows), Wo on its input dim
   (columns), pos_bias_u/v on the head axis.
 - Each core computes a partial out-projection (its 512 of 1024 contraction
   dims); the host sums the two partials per batch and adds bo.
 - probs slices concatenate with no reduction.

Per-core pipeline (all matmuls contract over the SBUF partition dim):
  1. PE-transpose W slices / hidden[b] / relpos so the contraction dim (model
     dim c) lands on partitions.
  2. QKV + pos projections as fp32r matmuls (1 cyc/row at N>=256).
  3. scores_ac = (q+u)/8 . k via matmul (two heads packed per issue slot with
     tile_position row groups, K=64 each).
  4. scores_bd: windowed matmul (q+v)/8 . p over a 1151-wide l-window per
     128-row q tile, bf16.
  5. Transformer-XL shift: write the [128,1151] window to a flat DRAM scratch,
     read it back with row stride 1150 at offset 127 - the diagonal
     shifted[p, k] = win[p, 127-p+k] becomes a regular strided DMA.
  6. scores = ac + shifted (DVE, PSUM+SBUF -> SBUF), exp in place on ACT with
     accum_out producing the softmax denominator for free.
  7. probs normalized to bf16 (ACT per-partition scale), DMA'd out with an
     fp32 cast (SWDGE), and PE-transposed for the ctx matmul.
  8. ctx = probsT.T @ v in bf16 (two heads packed via column tiling), then the
     out projection in fp32r.
"""

import os
import numpy as np
from contextlib import ExitStack

import concourse.bass as bass
import concourse.mybir as mybir
import concourse.tile as tile
from concourse import bacc
from concourse.bass_utils import run_bass_kernel_spmd
from concourse.masks import make_identity

F32 = mybir.dt.float32
F32R = mybir.dt.float32r
BF16 = mybir.dt.bfloat16

B, T, D, H = 4, 1024, 1024, 16
HS = D // H          # 64
L = 2 * T - 1        # 2047
HPC = 8              # heads per core
DPC = HPC * HS       # 512 dims per core
P = 128
NG = T // P          # 8 q/tok tiles
NCC = D // P         # 8 contraction chunks of the model dim
ND = DPC // P        # 4 chunks of this core's head dims
WIN = 1151           # l-window per q tile: 1024 + 127
SCALE = 1.0 / 8.0    # 1/sqrt(HS)

INV_SQRT = SCALE


def r32(ap):
    return ap.bitcast(F32R)


def build_program(nc, reps=1):
    # ---------------- I/O ----------------
    hid = nc.dram_tensor("hid", [T, D], F32, kind="ExternalInput").ap()
    rpe = nc.dram_tensor("rpe", [L, D], F32, kind="ExternalInput").ap()
    wq = nc.dram_tensor("wq", [DPC, D], F32, kind="ExternalInput").ap()
    wk = nc.dram_tensor("wk", [DPC, D], F32, kind="ExternalInput").ap()
    wv = nc.dram_tensor("wv", [DPC, D], F32, kind="ExternalInput").ap()
    wpos = nc.dram_tensor("wpos", [DPC, D], F32, kind="ExternalInput").ap()
    wo = nc.dram_tensor("wo", [D, DPC], F32, kind="ExternalInput").ap()
    bq = nc.dram_tensor("bq", [DPC], F32, kind="ExternalInput").ap()
    bk = nc.dram_tensor("bk", [DPC], F32, kind="ExternalInput").ap()
    bv = nc.dram_tensor("bv", [DPC], F32, kind="ExternalInput").ap()
    pbu = nc.dram_tensor("pbu", [DPC], F32, kind="ExternalInput").ap()
    pbv = nc.dram_tensor("pbv", [DPC], F32, kind="ExternalInput").ap()
    probs_o = nc.dram_tensor("probs_o", [HPC, T, T], F32, kind="ExternalOutput").ap()
    out_o = nc.dram_tensor("out_o", [T, D], F32, kind="ExternalOutput").ap()

    with tile.TileContext(nc) as tc:
      for _rep in range(reps):
       with ExitStack() as ctx:
        cpool = ctx.enter_context(tc.tile_pool(name="const", bufs=1))
        ident = cpool.tile([P, P], F32)
        make_identity(nc, ident)
        ident_bf = cpool.tile([P, P], BF16)
        make_identity(nc, ident_bf)

        # per-(d-chunk) bias columns: [128, ND]
        bq_sb = cpool.tile([P, ND], F32)
        bk_sb = cpool.tile([P, ND], F32)
        bv_sb = cpool.tile([P, ND], F32)
        pbu_sb = cpool.tile([P, ND], F32)
        pbv_sb = cpool.tile([P, ND], F32)
        for j in range(ND):
            for src, dst in ((bq, bq_sb), (bk, bk_sb), (bv, bv_sb),
                             (pbu, pbu_sb), (pbv, pbv_sb)):
                nc.sync.dma_start(dst[:, j:j + 1],
                                  src[P * j:P * (j + 1)].rearrange("(p o) -> p o", o=1))
        u_tot = cpool.tile([P, ND], F32)
        v_tot = cpool.tile([P, ND], F32)
        nc.vector.tensor_add(u_tot, bq_sb, pbu_sb)
        nc.vector.tensor_add(v_tot, bq_sb, pbv_sb)

        pT = cpool.tile([P, ND, 2048], BF16)
        quT = cpool.tile([P, ND, T], F32R)  # (q + bq + u) / 8, [d-part, tok]
        qvT = cpool.tile([P, ND, T], BF16)  # (q + bq + v) / 8
        kT = cpool.tile([P, ND, T], F32R)   # k + bk
        v_sb = cpool.tile([P, NG, DPC], BF16)  # v + bv, [tok-part, ktile, d]

        # attention-phase PSUM pools allocated first so the long-latency
        # shift pipeline can start before the setup pools release
        bd_ps = ctx.enter_context(tc.tile_pool(name="bd_ps", bufs=1, space="PSUM"))
        ac_ps = ctx.enter_context(tc.tile_pool(name="ac_ps", bufs=1, space="PSUM"))

        # ============ setup phase: pos + qkv projections ============
        with tc.tile_pool(name="proj_ps", bufs=2, space="PSUM") as proj_ps:
          with tc.tile_pool(name="pos_setup", bufs=2) as spool, \
               tc.tile_pool(name="pos_hold", bufs=1) as hpool, \
               tc.tile_pool(name="tr_ps", bufs=1, space="PSUM") as tr_ps:
            # wposT bf16 [c-part, NCC, DPC]
            wposT = hpool.tile([P, NCC, DPC], BF16)
            for jt in range(ND):
                wst = spool.tile([P, D], F32, tag="wst")
                nc.sync.dma_start(wst, wpos[P * jt:P * (jt + 1), :])
                wstb = spool.tile([P, D], BF16, tag="wstb")
                nc.scalar.copy(wstb, wst)
                for jh in range(2):
                    pst = tr_ps.tile([P, 512], BF16, tag="trb")
                    for i in range(4):
                        jc = 4 * jh + i
                        nc.tensor.transpose(pst[:, P * i:P * (i + 1)],
                                            wstb[:, P * jc:P * (jc + 1)], ident_bf)
                    nc.scalar.copy(wposT[:, 4 * jh:4 * jh + 4, P * jt:P * (jt + 1)],
                                   pst.rearrange("p (a b) -> p a b", a=4))
            # relposT bf16 [c-part, NCC, L]
            rpT = hpool.tile([P, NCC, 2048], BF16)
            for lt in range(16):
                rows = min(P, L - P * lt)  # last tile has 127 rows
                rst = spool.tile([P, D], F32, tag="rst")
                nc.sync.dma_start(rst[:rows], rpe[P * lt:P * lt + rows, :])
                rstb = spool.tile([P, D], BF16, tag="rstb")
                nc.scalar.copy(rstb[:rows], rst[:rows])
                for jh in range(2):
                    pst = tr_ps.tile([P, 512], BF16, tag="trb")
                    for i in range(4):
                        jc = 4 * jh + i
                        nc.tensor.transpose(pst[:, P * i:P * i + rows],
                                            rstb[:rows, P * jc:P * (jc + 1)],
                                            ident_bf[:rows, :rows])
                    nc.scalar.copy(
                        rpT[:, 4 * jh:4 * jh + 4, P * lt:P * lt + rows],
                        pst.rearrange("p (a b) -> p a b", a=4)[:, :, :rows])
            # pT[d, l] = sum_c wposT[c, d] * relposT[c, l]
            for jd in range(ND):
                for nl in range(4):
                    off = 512 * nl
                    nw = min(512, L - off)
                    ps = proj_ps.tile([P, 512], F32, tag="mm")
                    for jc in range(NCC):
                        nc.tensor.matmul(ps[:, :nw],
                                         wposT[:, jc, P * jd:P * (jd + 1)],
                                         rpT[:, jc, off:off + nw],
                                         start=(jc == 0), stop=(jc == NCC - 1))
                    nc.scalar.copy(pT[:, jd, off:off + nw], ps[:, :nw])

          with tc.tile_pool(name="qkv_setup", bufs=2) as spool, \
               tc.tile_pool(name="qkv_hold", bufs=1) as hpool, \
               tc.tile_pool(name="tr_ps2", bufs=1, space="PSUM") as tr_ps:
            # hiddenT [c-part, NCC, T] f32r
            hidT = hpool.tile([P, NCC, T], F32R)
            for gt in range(NG):
                hst = spool.tile([P, D], F32, tag="hst")
                nc.sync.dma_start(hst, hid[P * gt:P * (gt + 1), :])
                for jh in range(2):
                    pst = tr_ps.tile([P, 512], F32, tag="trf")
                    for i in range(4):
                        jc = 4 * jh + i
                        nc.tensor.transpose(pst[:, P * i:P * (i + 1)],
                                            hst[:, P * jc:P * (jc + 1)], ident)
                    nc.scalar.copy(hidT[:, 4 * jh:4 * jh + 4, P * gt:P * (gt + 1)],
                                   pst.rearrange("p (a b) -> p a b", a=4))

            vT_bf = hpool.tile([P, ND, T], BF16)
            for wi, wdram in enumerate((wq, wk, wv)):
                wT = spool.tile([P, NCC, DPC], F32R, tag="wT", bufs=2)
                for jt in range(ND):
                    wst = spool.tile([P, D], F32, tag="wst")
                    nc.sync.dma_start(wst, wdram[P * jt:P * (jt + 1), :])
                    for jh in range(2):
                        pst = tr_ps.tile([P, 512], F32, tag="trf")
                        for i in range(4):
                            jc = 4 * jh + i
                            nc.tensor.transpose(pst[:, P * i:P * (i + 1)],
                                                wst[:, P * jc:P * (jc + 1)], ident)
                        nc.scalar.copy(wT[:, 4 * jh:4 * jh + 4, P * jt:P * (jt + 1)],
                                       pst.rearrange("p (a b) -> p a b", a=4))
                for jd in range(ND):
                    for nt in range(2):
                        ps = proj_ps.tile([P, 512], F32, tag="mm")
                        for jc in range(NCC):
                            nc.tensor.matmul(ps,
                                             wT[:, jc, P * jd:P * (jd + 1)],
                                             hidT[:, jc, 512 * nt:512 * (nt + 1)],
                                             start=(jc == 0), stop=(jc == NCC - 1))
                        dst = slice(512 * nt, 512 * (nt + 1))
                        if wi == 0:
                            nc.vector.tensor_scalar(quT[:, jd, dst], ps,
                                                    u_tot[:, jd:jd + 1], SCALE,
                                                    mybir.AluOpType.add,
                                                    mybir.AluOpType.mult)
                            nc.vector.tensor_scalar(qvT[:, jd, dst], ps,
                                                    v_tot[:, jd:jd + 1], SCALE,
                                                    mybir.AluOpType.add,
                                                    mybir.AluOpType.mult)
                        elif wi == 1:
                            nc.vector.tensor_scalar(kT[:, jd, dst], ps,
                                                    bk_sb[:, jd:jd + 1], None,
                                                    mybir.AluOpType.add)
                        else:
                            nc.vector.tensor_scalar(vT_bf[:, jd, dst], ps,
                                                    bv_sb[:, jd:jd + 1], None,
                                                    mybir.AluOpType.add)
            # v natural layout [tok-part, ktile, d]
            for gt in range(NG):
                pst = tr_ps.tile([P, 512], BF16, tag="trf")
                for jd in range(ND):
                    nc.tensor.transpose(pst[:, P * jd:P * (jd + 1)],
                                        vT_bf[:, jd, P * gt:P * (gt + 1)],
                                        ident_bf)
                nc.scalar.copy(v_sb[:, gt, :], pst)

        # ============ attention ============
        Z = cpool.tile([P, HPC, NG], F32)
        pn_pool = ctx.enter_context(tc.tile_pool(name="pnorm", bufs=9))
        pt_pool = ctx.enter_context(tc.tile_pool(name="probsT", bufs=3))
        ctx_sb = ctx.enter_context(tc.tile_pool(name="ctxT", bufs=4))
        with tc.tile_pool(name="bd", bufs=4) as bd_pool, \
             tc.tile_pool(name="shift", bufs=4) as sh_pool, \
             tc.tile_pool(name="scores", bufs=3) as sc_pool, \
             tc.tile_pool(name="zrec", bufs=4) as zr_pool, \
             tc.tile_pool(name="dscratch", bufs=6, space="DRAM") as dram_pool, \
             tc.tile_pool(name="trb_ps", bufs=2, space="PSUM") as trb_ps, \
             tc.tile_pool(name="ctx_ps", bufs=1, space="PSUM") as ctx_ps:
            ctxT = []  # per pair: [128 d, T] f32r
            for pair in range(ND):
                probsT_tiles = {}
                for e in range(2):
                    h = 2 * pair + e
                    rg = slice(64 * e, 64 * (e + 1))
                    pnorm_h = []
                    for g in range(NG):
                        # ---- bd window matmul (bf16) ----
                        w0 = 896 - P * g
                        psb = bd_ps.tile([P, 1152], F32, tag="bd")
                        for off, nw in ((0, 512), (512, 512), (1024, 127)):
                            nc.tensor.matmul(psb[:, off:off + nw],
                                             qvT[rg, pair, P * g:P * (g + 1)],
                                             pT[rg, pair, w0 + off:w0 + off + nw],
                                             start=True, stop=True,
                                             tile_position=(64 * e, 0))
                        bd_sb = bd_pool.tile([P, 1152], BF16, tag="bd")
                        nc.scalar.copy(bd_sb[:, :WIN], psb[:, :WIN])
                        # ---- shift via DRAM round trip ----
                        if os.environ.get("K_NOSHIFT"):
                            shifted = bd_sb[:, :T]
                        else:
                            dsc = dram_pool.tile([P * WIN], BF16, tag="dsc")
                            nc.scalar.dma_start(dsc.rearrange("(p j) -> p j", j=WIN),
                                                bd_sb[:, :WIN])
                            shifted = sh_pool.tile([P, T], BF16, tag="sh")
                            diag = bass.AP(dsc.tensor, 127, [[WIN - 1, P], [1, T]])
                            nc.sync.dma_start(shifted, diag)
                        # ---- ac matmul; psum freed fast, shift added later ----
                        psa = ac_ps.tile([P, T], F32, tag="ac")
                        for n2 in range(2):
                            nc.tensor.matmul(psa[:, 512 * n2:512 * (n2 + 1)],
                                             quT[rg, pair, P * g:P * (g + 1)],
                                             kT[rg, pair, 512 * n2:512 * (n2 + 1)],
                                             start=True, stop=True,
                                             tile_position=(64 * e, 0))
                        scores = sc_pool.tile([P, T], F32, tag="sc")
                        nc.vector.tensor_copy(scores, psa)
                        nc.vector.tensor_add(scores, scores, shifted)
                        # ---- softmax ----
                        nc.scalar.activation(scores, scores,
                                             mybir.ActivationFunctionType.Exp,
                                             accum_out=Z[:, h, g:g + 1])
                        zrec = zr_pool.tile([P, 1], F32, tag="zr")
                        nc.vector.reciprocal(zrec, Z[:, h, g:g + 1])
                        pnorm = pn_pool.tile([P, T], BF16, tag="pn")
                        nc.gpsimd.tensor_scalar(pnorm, scores, zrec, None,
                                                mybir.AluOpType.mult)
                        if not os.environ.get("K_NOPROBS"):
                            nc.gpsimd.dma_start(probs_o[h, P * g:P * (g + 1), :],
                                                pnorm)
                        pnorm_h.append(pnorm)
                    # ---- transpose probs for ctx ----
                    probsT = pt_pool.tile([P, NG, T], BF16, tag="pt")
                    for kt in range(NG):
                        for quad in range(2):
                            pst = trb_ps.tile([P, 512], BF16, tag="trq")
                            for gq in range(4):
                                g = 4 * quad + gq
                                nc.tensor.transpose(pst[:, P * gq:P * (gq + 1)],
                                                    pnorm_h[g][:, P * kt:P * (kt + 1)],
                                                    ident_bf)
                            nc.vector.tensor_copy(
                                probsT[:, kt, 512 * quad:512 * (quad + 1)], pst)
                    probsT_tiles[e] = probsT
                # ---- ctx matmul: ctxT[d, tok] ----
                cT = ctx_sb.tile([P, T], F32R, tag="ctxT")
                for n2 in range(2):
                    ps = ctx_ps.tile([P, 512], F32, tag="ctx")
                    for e in range(2):
                        for kt in range(NG):
                            nc.tensor.matmul(
                                ps[64 * e:64 * (e + 1), :],
                                v_sb[:, kt,
                                     P * pair + 64 * e:P * pair + 64 * (e + 1)],
                                probsT_tiles[e][:, kt, 512 * n2:512 * (n2 + 1)],
                                start=(kt == 0), stop=(kt == NG - 1),
                                tile_position=(0, 64 * e))
                    nc.vector.tensor_copy(cT[:, 512 * n2:512 * (n2 + 1)], ps)
                ctxT.append(cT)

        # ============ out projection ============
        with tc.tile_pool(name="wo_setup", bufs=2) as spool, \
             tc.tile_pool(name="wo_hold", bufs=1) as hpool, \
             tc.tile_pool(name="tr_ps3", bufs=1, space="PSUM") as tr_ps, \
             tc.tile_pool(name="out_ps", bufs=2, space="PSUM") as out_ps, \
             tc.tile_pool(name="osb", bufs=2) as opool:
            woT = hpool.tile([P, ND, D], F32R)  # [d-part, chunk, outdim]
            for ot in range(NG):
                wst = spool.tile([P, DPC], F32, tag="wost")
                pst = tr_ps.tile([P, 512], F32, tag="trf3")
                nc.sync.dma_start(wst, wo[P * ot:P * (ot + 1), :])
                for jd in range(ND):
                    nc.tensor.transpose(pst[:, P * jd:P * (jd + 1)],
                                        wst[:, P * jd:P * (jd + 1)], ident)
                nc.scalar.copy(woT[:, :, P * ot:P * (ot + 1)],
                               pst.rearrange("p (a b) -> p a b", a=4))
            for gt in range(NG):
                osb = opool.tile([P, D], F32, tag="osb")
                for n2 in range(2):
                    ps = out_ps.tile([P, 512], F32, tag="out")
                    for jd in range(ND):
                        nc.tensor.matmul(ps,
                                         ctxT[jd][:, P * gt:P * (gt + 1)],
                                         woT[:, jd, 512 * n2:512 * (n2 + 1)],
                                         start=(jd == 0), stop=(jd == ND - 1))
                    nc.scalar.copy(osb[:, 512 * n2:512 * (n2 + 1)], ps)
                nc.sync.dma_start(out_o[P * gt:P * (gt + 1), :], osb)


_CACHE = {}


def get_compiled():
    if "nc" not in _CACHE:
        nc = bacc.Bacc("TRN2", target_bir_lowering=False, debug=False,
                       enable_asserts=False, num_devices=8)
        build_program(nc)
        nc.compile()
        _CACHE["nc"] = nc
    return _CACHE["nc"]


def shard_inputs(inputs):
    hs = np.asarray(inputs["hidden_states"], np.float32)
    rpe = np.asarray(inputs["relative_position_embeddings"], np.float32)[0]
    Wq = np.asarray(inputs["Wq"], np.float32)
    Wk = np.asarray(inputs["Wk"], np.float32)
    Wv = np.asarray(inputs["Wv"], np.float32)
    Wo = np.asarray(inputs["Wo"], np.float32)
    Wpos = np.asarray(inputs["Wpos"], np.float32)
    bq = np.asarray(inputs["bq"], np.float32)
    bk = np.asarray(inputs["bk"], np.float32)
    bv = np.asarray(inputs["bv"], np.float32)
    pbu = np.asarray(inputs["pos_bias_u"], np.float32).reshape(-1)
    pbv = np.asarray(inputs["pos_bias_v"], np.float32).reshape(-1)
    maps = []
    for c in range(8):
        b, hg = c % 4, c // 4
        s = slice(DPC * hg, DPC * (hg + 1))
        maps.append({
            "hid": np.ascontiguousarray(hs[b]),
            "rpe": np.ascontiguousarray(rpe),
            "wq": np.ascontiguousarray(Wq[s]),
            "wk": np.ascontiguousarray(Wk[s]),
            "wv": np.ascontiguousarray(Wv[s]),
            "wpos": np.ascontiguousarray(Wpos[s]),
            "wo": np.ascontiguousarray(Wo[:, s]),
            "bq": np.ascontiguousarray(bq[s]),
            "bk": np.ascontiguousarray(bk[s]),
            "bv": np.ascontiguousarray(bv[s]),
            "pbu": np.ascontiguousarray(pbu[s]),
            "pbv": np.ascontiguousarray(pbv[s]),
        })
    return maps


def assemble_outputs(results, bo):
    out = np.zeros((B, T, D), np.float32)
    probs = np.empty((B, H, T, T), np.float32)
    for c, r in enumerate(results):
        b, hg = c % 4, c // 4
        out[b] += r["out_o"]
        probs[b, HPC * hg:HPC * (hg + 1)] = r["probs_o"]
    out += np.asarray(bo, np.float32)
    return out, probs


def kernel(**inputs):
    nc = get_compiled()
    in_maps = shard_inputs(inputs)
    res = run_bass_kernel_spmd(nc, in_maps, core_ids=list(range(8)))
    _CACHE["last_results"] = res
    return assemble_outputs(res.results, inputs["bo"])


# revision 20
# speedup vs baseline: 1.6740x; 1.6740x over previous
"""Conformer (YMT3) relative-position self-attention on 8 Trainium2 cores.

Full-input contract: kernel(**inputs) takes the unsharded tensors and returns
(out, probs) exactly like the reference.

Sharding: core c handles batch (c % 4) and heads [8*(c//4), 8*(c//4)+8).
 - Wq/Wk/Wv/Wpos are sharded on their output dim (rows), Wo on its input dim
   (columns), pos_bias_u/v on the head axis.
 - Each core computes a partial out-projection (its 512 of 1024 contraction
   dims); the host sums the two partials per batch and adds bo.
 - probs slices concatenate with no reduction.

Per-core pipeline (all matmuls contract over the SBUF partition dim):
  1. PE-transpose W slices / hidden[b] / relpos so the contraction dim (model
     dim c) lands on partitions.
  2. QKV + pos projections as fp32r matmuls (1 cyc/row at N>=256).
  3. scores_ac = (q+u)/8 . k via matmul (two heads packed per issue slot with
     tile_position row groups, K=64 each).
  4. scores_bd: windowed matmul (q+v)/8 . p over a 1151-wide l-window per
     128-row q tile, bf16.
  5. Transformer-XL shift: write the [128,1151] window to a flat DRAM scratch,
     read it back with row stride 1150 at offset 127 - the diagonal
     shifted[p, k] = win[p, 127-p+k] becomes a regular strided DMA.
  6. scores = ac + shifted (DVE, PSUM+SBUF -> SBUF), exp in place on ACT with
     accum_out producing the softmax denominator for free.
  7. probs normalized to bf16 on GPSIMD (per-partition 1/Z scale), written out
     per head with a single fp32-casting SWDGE DMA, and transposed for the ctx
     matmul via a DRAM round trip + XBAR dma_start_transpose (bf16).
  8. ctx = probsT.T @ v in bf16 (two heads packed via column tiling), then the
     out projection in fp32r.

Timing note: under this axon tunnel no NTFF profiling is available and
per-call dispatch overhead (~2.5-5 ms) swamps the kernel, so test.py measures
device time by building the same body with reps=1 and reps=5 and taking the
slope; build_program(nc, reps=N) exists for that purpose.
"""

import os
import numpy as np
from contextlib import ExitStack

import concourse.bass as bass
import concourse.mybir as mybir
import concourse.tile as tile
from concourse import bacc
from concourse.bass_utils import run_bass_kernel_spmd
from concourse.masks import make_identity

F32 = mybir.dt.float32
F32R = mybir.dt.float32r
BF16 = mybir.dt.bfloat16

B, T, D, H = 4, 1024, 1024, 16
HS = D // H          # 64
L = 2 * T - 1        # 2047
HPC = 8              # heads per core
DPC = HPC * HS       # 512 dims per core
P = 128
NG = T // P          # 8 q/tok tiles
NCC = D // P         # 8 contraction chunks of the model dim
ND = DPC // P        # 4 chunks of this core's head dims
WIN = 1151           # l-window per q tile: 1024 + 127
SCALE = 1.0 / 8.0    # 1/sqrt(HS)

INV_SQRT = SCALE


def r32(ap):
    return ap.bitcast(F32R)


def build_program(nc, reps=1):
    # ---------------- I/O ----------------
    hid = nc.dram_tensor("hid", [T, D], F32, kind="ExternalInput").ap()
    rpe = nc.dram_tensor("rpe", [L, D], F32, kind="ExternalInput").ap()
    wq = nc.dram_tensor("wq", [DPC, D], F32, kind="ExternalInput").ap()
    wk = nc.dram_tensor("wk", [DPC, D], F32, kind="ExternalInput").ap()
    wv = nc.dram_tensor("wv", [DPC, D], F32, kind="ExternalInput").ap()
    wpos = nc.dram_tensor("wpos", [DPC, D], F32, kind="ExternalInput").ap()
    wo = nc.dram_tensor("wo", [D, DPC], F32, kind="ExternalInput").ap()
    bq = nc.dram_tensor("bq", [DPC], F32, kind="ExternalInput").ap()
    bk = nc.dram_tensor("bk", [DPC], F32, kind="ExternalInput").ap()
    bv = nc.dram_tensor("bv", [DPC], F32, kind="ExternalInput").ap()
    pbu = nc.dram_tensor("pbu", [DPC], F32, kind="ExternalInput").ap()
    pbv = nc.dram_tensor("pbv", [DPC], F32, kind="ExternalInput").ap()
    probs_o = nc.dram_tensor("probs_o", [HPC, T, T], F32, kind="ExternalOutput").ap()
    out_o = nc.dram_tensor("out_o", [T, D], F32, kind="ExternalOutput").ap()

    with tile.TileContext(nc) as tc:
      for _rep in range(reps):
       with ExitStack() as ctx:
        cpool = ctx.enter_context(tc.tile_pool(name="const", bufs=1))
        ident = cpool.tile([P, P], F32)
        make_identity(nc, ident)
        ident_bf = cpool.tile([P, P], BF16)
        make_identity(nc, ident_bf)

        # per-(d-chunk) bias columns: [128, ND]
        bq_sb = cpool.tile([P, ND], F32)
        bk_sb = cpool.tile([P, ND], F32)
        bv_sb = cpool.tile([P, ND], F32)
        pbu_sb = cpool.tile([P, ND], F32)
        pbv_sb = cpool.tile([P, ND], F32)
        for j in range(ND):
            for src, dst in ((bq, bq_sb), (bk, bk_sb), (bv, bv_sb),
                             (pbu, pbu_sb), (pbv, pbv_sb)):
                nc.sync.dma_start(dst[:, j:j + 1],
                                  src[P * j:P * (j + 1)].rearrange("(p o) -> p o", o=1))
        u_tot = cpool.tile([P, ND], F32)
        v_tot = cpool.tile([P, ND], F32)
        nc.vector.tensor_add(u_tot, bq_sb, pbu_sb)
        nc.vector.tensor_add(v_tot, bq_sb, pbv_sb)

        pT = cpool.tile([P, ND, 2048], BF16)
        quT = cpool.tile([P, ND, T], F32R)  # (q + bq + u) / 8, [d-part, tok]
        qvT = cpool.tile([P, ND, T], BF16)  # (q + bq + v) / 8
        kT = cpool.tile([P, ND, T], F32R)   # k + bk
        v_sb = cpool.tile([P, NG, DPC], BF16)  # v + bv, [tok-part, ktile, d]

        # attention-phase PSUM pools allocated first so the long-latency
        # shift pipeline can start before the setup pools release
        bd_ps = ctx.enter_context(tc.tile_pool(name="bd_ps", bufs=1, space="PSUM"))
        ac_ps = ctx.enter_context(tc.tile_pool(name="ac_ps", bufs=1, space="PSUM"))

        # ============ setup phase: pos + qkv projections ============
        with tc.tile_pool(name="proj_ps", bufs=2, space="PSUM") as proj_ps:
          with tc.tile_pool(name="pos_setup", bufs=2) as spool, \
               tc.tile_pool(name="pos_hold", bufs=1) as hpool, \
               tc.tile_pool(name="tr_ps", bufs=1, space="PSUM") as tr_ps:
            # wposT bf16 [c-part, NCC, DPC]
            wposT = hpool.tile([P, NCC, DPC], BF16)
            for jt in range(ND):
                wst = spool.tile([P, D], F32, tag="wst")
                nc.sync.dma_start(wst, wpos[P * jt:P * (jt + 1), :])
                wstb = spool.tile([P, D], BF16, tag="wstb")
                nc.scalar.copy(wstb, wst)
                for jh in range(2):
                    pst = tr_ps.tile([P, 512], BF16, tag="trb")
                    for i in range(4):
                        jc = 4 * jh + i
                        nc.tensor.transpose(pst[:, P * i:P * (i + 1)],
                                            wstb[:, P * jc:P * (jc + 1)], ident_bf)
                    nc.scalar.copy(wposT[:, 4 * jh:4 * jh + 4, P * jt:P * (jt + 1)],
                                   pst.rearrange("p (a b) -> p a b", a=4))
            # relposT bf16 [c-part, NCC, L]
            rpT = hpool.tile([P, NCC, 2048], BF16)
            for lt in range(16):
                rows = min(P, L - P * lt)  # last tile has 127 rows
                rst = spool.tile([P, D], F32, tag="rst")
                nc.sync.dma_start(rst[:rows], rpe[P * lt:P * lt + rows, :])
                rstb = spool.tile([P, D], BF16, tag="rstb")
                nc.scalar.copy(rstb[:rows], rst[:rows])
                for jh in range(2):
                    pst = tr_ps.tile([P, 512], BF16, tag="trb")
                    for i in range(4):
                        jc = 4 * jh + i
                        nc.tensor.transpose(pst[:, P * i:P * i + rows],
                                            rstb[:rows, P * jc:P * (jc + 1)],
                                            ident_bf[:rows, :rows])
                    nc.scalar.copy(
                        rpT[:, 4 * jh:4 * jh + 4, P * lt:P * lt + rows],
                        pst.rearrange("p (a b) -> p a b", a=4)[:, :, :rows])
            # pT[d, l] = sum_c wposT[c, d] * relposT[c, l]
            for jd in range(ND):
                for nl in range(4):
                    off = 512 * nl
                    nw = min(512, L - off)
                    ps = proj_ps.tile([P, 512], F32, tag="mm")
                    for jc in range(NCC):
                        nc.tensor.matmul(ps[:, :nw],
                                         wposT[:, jc, P * jd:P * (jd + 1)],
                                         rpT[:, jc, off:off + nw],
                                         start=(jc == 0), stop=(jc == NCC - 1))
                    nc.scalar.copy(pT[:, jd, off:off + nw], ps[:, :nw])

          with tc.tile_pool(name="qkv_setup", bufs=2) as spool, \
               tc.tile_pool(name="qkv_hold", bufs=1) as hpool, \
               tc.tile_pool(name="tr_ps2", bufs=1, space="PSUM") as tr_ps:
            # hiddenT [c-part, NCC, T] f32r
            hidT = hpool.tile([P, NCC, T], F32R)
            for gt in range(NG):
                hst = spool.tile([P, D], F32, tag="hst")
                nc.sync.dma_start(hst, hid[P * gt:P * (gt + 1), :])
                for jh in range(2):
                    pst = tr_ps.tile([P, 512], F32, tag="trf")
                    for i in range(4):
                        jc = 4 * jh + i
                        nc.tensor.transpose(pst[:, P * i:P * (i + 1)],
                                            hst[:, P * jc:P * (jc + 1)], ident)
                    nc.scalar.copy(hidT[:, 4 * jh:4 * jh + 4, P * gt:P * (gt + 1)],
                                   pst.rearrange("p (a b) -> p a b", a=4))

            vT_bf = hpool.tile([P, ND, T], BF16)
            for wi, wdram in enumerate((wq, wk, wv)):
                wT = spool.tile([P, NCC, DPC], F32R, tag="wT", bufs=2)
                for jt in range(ND):
                    wst = spool.tile([P, D], F32, tag="wst")
                    nc.sync.dma_start(wst, wdram[P * jt:P * (jt + 1), :])
                    for jh in range(2):
                        pst = tr_ps.tile([P, 512], F32, tag="trf")
                        for i in range(4):
                            jc = 4 * jh + i
                            nc.tensor.transpose(pst[:, P * i:P * (i + 1)],
                                                wst[:, P * jc:P * (jc + 1)], ident)
                        nc.scalar.copy(wT[:, 4 * jh:4 * jh + 4, P * jt:P * (jt + 1)],
                                       pst.rearrange("p (a b) -> p a b", a=4))
                for jd in range(ND):
                    for nt in range(2):
                        ps = proj_ps.tile([P, 512], F32, tag="mm")
                        for jc in range(NCC):
                            nc.tensor.matmul(ps,
                                             wT[:, jc, P * jd:P * (jd + 1)],
                                             hidT[:, jc, 512 * nt:512 * (nt + 1)],
                                             start=(jc == 0), stop=(jc == NCC - 1))
                        dst = slice(512 * nt, 512 * (nt + 1))
                        if wi == 0:
                            nc.vector.tensor_scalar(quT[:, jd, dst], ps,
                                                    u_tot[:, jd:jd + 1], SCALE,
                                                    mybir.AluOpType.add,
                                                    mybir.AluOpType.mult)
                            nc.vector.tensor_scalar(qvT[:, jd, dst], ps,
                                                    v_tot[:, jd:jd + 1], SCALE,
                                                    mybir.AluOpType.add,
                                                    mybir.AluOpType.mult)
                        elif wi == 1:
                            nc.vector.tensor_scalar(kT[:, jd, dst], ps,
                                                    bk_sb[:, jd:jd + 1], None,
                                                    mybir.AluOpType.add)
                        else:
                            nc.vector.tensor_scalar(vT_bf[:, jd, dst], ps,
                                                    bv_sb[:, jd:jd + 1], None,
                                                    mybir.AluOpType.add)
            # v natural layout [tok-part, ktile, d]
            for gt in range(NG):
                pst = tr_ps.tile([P, 512], BF16, tag="trf")
                for jd in range(ND):
                    nc.tensor.transpose(pst[:, P * jd:P * (jd + 1)],
                                        vT_bf[:, jd, P * gt:P * (gt + 1)],
                                        ident_bf)
                nc.scalar.copy(v_sb[:, gt, :], pst)

        # ============ attention ============
        Z = cpool.tile([P, HPC, NG], F32)
        pn_pool = ctx.enter_context(tc.tile_pool(name="pnorm", bufs=3))
        pt_pool = ctx.enter_context(tc.tile_pool(name="probsT", bufs=18))
        ctx_sb = ctx.enter_context(tc.tile_pool(name="ctxT", bufs=4))
        with tc.tile_pool(name="bd", bufs=6) as bd_pool, \
             tc.tile_pool(name="shift", bufs=6) as sh_pool, \
             tc.tile_pool(name="scores", bufs=4) as sc_pool, \
             tc.tile_pool(name="zrec", bufs=4) as zr_pool, \
             tc.tile_pool(name="dscratch", bufs=10, space="DRAM") as dram_pool, \
             tc.tile_pool(name="ctx_ps", bufs=1, space="PSUM") as ctx_ps:
            ctxT = []  # per pair: [128 d, T] f32r
            for pair in range(ND):
                probsT_tiles = {}
                for e in range(2):
                    h = 2 * pair + e
                    rg = slice(64 * e, 64 * (e + 1))
                    pnorm_h = pn_pool.tile([P, NG, T], BF16, tag="pn")
                    for g in range(NG):
                        # ---- bd window matmul (bf16) ----
                        w0 = 896 - P * g
                        psb = bd_ps.tile([P, 1152], F32, tag="bd")
                        for off, nw in ((0, 512), (512, 512), (1024, 127)):
                            nc.tensor.matmul(psb[:, off:off + nw],
                                             qvT[rg, pair, P * g:P * (g + 1)],
                                             pT[rg, pair, w0 + off:w0 + off + nw],
                                             start=True, stop=True,
                                             tile_position=(64 * e, 0))
                        bd_sb = bd_pool.tile([P, 1152], BF16, tag="bd")
                        nc.scalar.copy(bd_sb[:, :WIN], psb[:, :WIN])
                        # ---- shift via DRAM round trip ----
                        if os.environ.get("K_NOSHIFT"):
                            shifted = bd_sb[:, :T]
                        else:
                            dsc = dram_pool.tile([P * WIN], BF16, tag="dsc")
                            nc.scalar.dma_start(dsc.rearrange("(p j) -> p j", j=WIN),
                                                bd_sb[:, :WIN])
                            shifted = sh_pool.tile([P, T], BF16, tag="sh")
                            diag = bass.AP(dsc.tensor, 127, [[WIN - 1, P], [1, T]])
                            nc.sync.dma_start(shifted, diag)
                        # ---- ac matmul; psum freed fast, shift added later ----
                        psa = ac_ps.tile([P, T], F32, tag="ac")
                        for n2 in range(2):
                            nc.tensor.matmul(psa[:, 512 * n2:512 * (n2 + 1)],
                                             quT[rg, pair, P * g:P * (g + 1)],
                                             kT[rg, pair, 512 * n2:512 * (n2 + 1)],
                                             start=True, stop=True,
                                             tile_position=(64 * e, 0))
                        scores = sc_pool.tile([P, T], F32, tag="sc")
                        nc.vector.tensor_copy(scores, psa)
                        nc.vector.tensor_add(scores, scores, shifted)
                        # ---- softmax ----
                        nc.scalar.activation(scores, scores,
                                             mybir.ActivationFunctionType.Exp,
                                             accum_out=Z[:, h, g:g + 1])
                        zrec = zr_pool.tile([P, 1], F32, tag="zr")
                        nc.vector.reciprocal(zrec, Z[:, h, g:g + 1])
                        nc.gpsimd.tensor_scalar(pnorm_h[:, g, :], scores, zrec,
                                                None, mybir.AluOpType.mult)
                    # ---- probs output: one cast DMA per head ----
                    if not os.environ.get("K_NOPROBS"):
                        nc.gpsimd.dma_start(
                            probs_o[h].rearrange("(g p) k -> p g k", p=P), pnorm_h)
                    # ---- transpose probs for ctx via DRAM + XBAR ----
                    pscr = dram_pool.tile([T * T], BF16, tag="pscr")
                    nc.sync.dma_start(
                        pscr.rearrange("(g p k) -> p g k", g=NG, p=P), pnorm_h)
                    pscr2d = pscr.rearrange("(q k) -> q k", k=T)
                    probsT = []
                    for kt in range(NG):
                        ptt = pt_pool.tile([P, T], BF16, tag="pt")
                        nc.sync.dma_start_transpose(
                            ptt, pscr2d[:, P * kt:P * (kt + 1)])
                        probsT.append(ptt)
                    probsT_tiles[e] = probsT
                # ---- ctx matmul: ctxT[d, tok] ----
                cT = ctx_sb.tile([P, T], F32R, tag="ctxT")
                for n2 in range(2):
                    ps = ctx_ps.tile([P, 512], F32, tag="ctx")
                    for e in range(2):
                        for kt in range(NG):
                            nc.tensor.matmul(
                                ps[64 * e:64 * (e + 1), :],
                                v_sb[:, kt,
                                     P * pair + 64 * e:P * pair + 64 * (e + 1)],
                                probsT_tiles[e][kt][:, 512 * n2:512 * (n2 + 1)],
                                start=(kt == 0), stop=(kt == NG - 1),
                                tile_position=(0, 64 * e))
                    nc.vector.tensor_copy(cT[:, 512 * n2:512 * (n2 + 1)], ps)
                ctxT.append(cT)

        # ============ out projection ============
        with tc.tile_pool(name="wo_setup", bufs=2) as spool, \
             tc.tile_pool(name="wo_hold", bufs=1) as hpool, \
             tc.tile_pool(name="tr_ps3", bufs=1, space="PSUM") as tr_ps, \
             tc.tile_pool(name="out_ps", bufs=2, space="PSUM") as out_ps, \
             tc.tile_pool(name="osb", bufs=2) as opool:
            woT = hpool.tile([P, ND, D], F32R)  # [d-part, chunk, outdim]
            for ot in range(NG):
                wst = spool.tile([P, DPC], F32, tag="wost")
                pst = tr_ps.tile([P, 512], F32, tag="trf3")
                nc.sync.dma_start(wst, wo[P * ot:P * (ot + 1), :])
                for jd in range(ND):
                    nc.tensor.transpose(pst[:, P * jd:P * (jd + 1)],
                                        wst[:, P * jd:P * (jd + 1)], ident)
                nc.scalar.copy(woT[:, :, P * ot:P * (ot + 1)],
                               pst.rearrange("p (a b) -> p a b", a=4))
            for gt in range(NG):
                osb = opool.tile([P, D], F32, tag="osb")
                for n2 in range(2):
                    ps = out_ps.tile([P, 512], F32, tag="out")
                    for jd in range(ND):
                        nc.tensor.matmul(ps,
                                         ctxT[jd][:, P * gt:P * (gt + 1)],
                                         woT[:, jd, 512 * n2:512 * (n2 + 1)],
                                         start=(jd == 0), stop=(jd == ND - 1))
                    nc.scalar.copy(osb[:, 512 * n2:512 * (n2 + 1)], ps)
                nc.sync.dma_start(out_o[P * gt:P * (gt + 1), :], osb)


_CACHE = {}


def get_compiled():
    if "nc" not in _CACHE:
        nc = bacc.Bacc("TRN2", target_bir_lowering=False, debug=False,
                       enable_asserts=False, num_devices=8)
        build_program(nc)
        nc.compile()
        _CACHE["nc"] = nc
    return _CACHE["nc"]


def shard_inputs(inputs):
    hs = np.asarray(inputs["hidden_states"], np.float32)
    rpe = np.asarray(inputs["relative_position_embeddings"], np.float32)[0]
    Wq = np.asarray(inputs["Wq"], np.float32)
    Wk = np.asarray(inputs["Wk"], np.float32)
    Wv = np.asarray(inputs["Wv"], np.float32)
    Wo = np.asarray(inputs["Wo"], np.float32)
    Wpos = np.asarray(inputs["Wpos"], np.float32)
    bq = np.asarray(inputs["bq"], np.float32)
    bk = np.asarray(inputs["bk"], np.float32)
    bv = np.asarray(inputs["bv"], np.float32)
    pbu = np.asarray(inputs["pos_bias_u"], np.float32).reshape(-1)
    pbv = np.asarray(inputs["pos_bias_v"], np.float32).reshape(-1)
    maps = []
    for c in range(8):
        b, hg = c % 4, c // 4
        s = slice(DPC * hg, DPC * (hg + 1))
        maps.append({
            "hid": np.ascontiguousarray(hs[b]),
            "rpe": np.ascontiguousarray(rpe),
            "wq": np.ascontiguousarray(Wq[s]),
            "wk": np.ascontiguousarray(Wk[s]),
            "wv": np.ascontiguousarray(Wv[s]),
            "wpos": np.ascontiguousarray(Wpos[s]),
            "wo": np.ascontiguousarray(Wo[:, s]),
            "bq": np.ascontiguousarray(bq[s]),
            "bk": np.ascontiguousarray(bk[s]),
            "bv": np.ascontiguousarray(bv[s]),
            "pbu": np.ascontiguousarray(pbu[s]),
            "pbv": np.ascontiguousarray(pbv[s]),
        })
    return maps


def assemble_outputs(results, bo):
    out = np.zeros((B, T, D), np.float32)
    probs = np.empty((B, H, T, T), np.float32)
    for c, r in enumerate(results):
        b, hg = c % 4, c // 4
        out[b] += r["out_o"]
        probs[b, HPC * hg:HPC * (hg + 1)] = r["probs_o"]
    out += np.asarray(bo, np.float32)
    return out, probs


def kernel(**inputs):
    nc = get_compiled()
    in_maps = shard_inputs(inputs)
    res = run_bass_kernel_spmd(nc, in_maps, core_ids=list(range(8)))
    _CACHE["last_results"] = res
    return assemble_outputs(res.results, inputs["bo"])
